# revision 1
# baseline (speedup 1.0000x reference)
"""Trainium2 Bass kernel for nn_CADense (context-adaptive low-rank dense layer).

Computes, for the full batch:
    s_mod = s + context @ w          # [B, R]
    low   = (data @ u) * s_mod       # [B, R]
    out   = relu(low @ v.T + 2*bias) # [B, UNITS]

Sharding: data-parallel over batch across 8 NeuronCores; u/s/v/w/bias
replicated. Each core runs the same Bass program on its 1024-row shard.

The PE contracts over the partition dim, so the big operands are marshaled
host-side into contraction-major layouts (data.T, context.T, v.T) when the
shards are built — on-chip PE transposes would otherwise dominate the
kernel. All matmuls run as float32r (full-rate fp32 streaming mode).

Compute is done in the "transposed" domain per rank-chunk:
    lowT[r, b] = (u.T @ data.T)[r, b] * (s[r] + (w.T @ ctx.T)[r, b])
with the s-add fused into the scalar-engine PSUM evacuation. The final
matmul returns to natural [b, units] layout; the 2*bias add is folded in
as a K=1 rank-1 matmul into the same PSUM accumulation group and ReLU
evacuation of the output PSUM groups alternates between the scalar and
vector engines so neither gates PSUM recycling.

Schedule notes:
- Input DMAs are spread across both HWDGE queues (sync: data tiles,
  scalar: weights/context) and output stores go through the gpsimd SWDGE
  queue — three independent descriptor rings so transfers overlap and
  the HBM link stays saturated.
- The two 512-row batch tiles are software-pipelined; PE emission
  interleaves batch-tile 1's rank stage with batch-tile 0's output stage
  and the (DMA-independent) context matmuls fill data-DMA wait bubbles,
  so the PE never idles long enough for the HAM clock gate to
  re-throttle.
- A short burst of bf16 dummy matmuls on garbage SBUF pre-warms the HAM
  clock gate while the first DMAs stream in.
"""

import os
import sys
from contextlib import ExitStack

import numpy as np


def _ensure_concourse():
    try:
        import concourse  # noqa: F401
    except ImportError:
        for p in ("/opt/trn_rl_repo", "/root/.axon_site/_ro/trn_rl_repo"):
            if os.path.isdir(p) and p not in sys.path:
                sys.path.insert(0, p)


_ensure_concourse()

import concourse.tile as tile  # noqa: E402
from concourse import bacc, mybir  # noqa: E402
from concourse.bass_utils import run_bass_kernel_spmd  # noqa: E402

NCORES = 8
B, N_IN, UNITS, RANK, CCTX = 8192, 2048, 2048, 256, 512
NB = B // NCORES  # batch rows per core
P = 128
BT = 512  # batch tile (free dim of T-domain matmuls)
NBT = NB // BT  # batch tiles per core
KC = N_IN // P  # 16 contraction chunks for data @ u
CC = CCTX // P  # 4 contraction chunks for context @ w
RC = RANK // P  # 2 rank chunks
MS = 512  # output units slice width
NMS = UNITS // MS  # 4 unit slices
N_WARMUP_MM = 14

F32 = mybir.dt.float32
F32R = mybir.dt.float32r
BF16 = mybir.dt.bfloat16


def _emit(nc, tc, ctx):
    # Host-marshaled transposed layouts: dataT = data.T, ctxT = context.T,
    # vT = v.T (built per-shard in kernel()).
    d_dataT = nc.dram_tensor("dataT", [N_IN, NB], F32R, kind="ExternalInput")
    d_ctxT = nc.dram_tensor("ctxT", [CCTX, NB], F32R, kind="ExternalInput")
    d_u = nc.dram_tensor("u", [N_IN, RANK], F32R, kind="ExternalInput")
    d_s = nc.dram_tensor("s", [RANK], F32, kind="ExternalInput")
    d_vT = nc.dram_tensor("vT", [RANK, UNITS], F32R, kind="ExternalInput")
    d_w = nc.dram_tensor("w", [CCTX, RANK], F32R, kind="ExternalInput")
    d_bias = nc.dram_tensor("bias", [UNITS], F32R, kind="ExternalInput")
    d_out = nc.dram_tensor("out", [NB, UNITS], F32, kind="ExternalOutput")

    ap_dataT = d_dataT.ap().rearrange("(q j p) b -> p q j b", p=P, j=4)
    ap_ctxT = d_ctxT.ap().rearrange("(cc p) b -> p cc b", p=P)
    ap_u = d_u.ap().rearrange("(uq j p) r -> p uq j r", p=P, j=4)
    ap_vT = d_vT.ap().rearrange("(rc p) m -> p rc m", p=P)

    singles = ctx.enter_context(tc.tile_pool(name="singles", bufs=1))
    du_psum = ctx.enter_context(tc.tile_pool(name="du_psum", bufs=2, space="PSUM"))
    s_psum = ctx.enter_context(tc.tile_pool(name="s_psum", bufs=2, space="PSUM"))
    o_psum = ctx.enter_context(tc.tile_pool(name="o_psum", bufs=4, space="PSUM"))
    dTpool = ctx.enter_context(tc.tile_pool(name="dataT", bufs=1))
    cTpool = ctx.enter_context(tc.tile_pool(name="ctxT", bufs=2))
    lowpool = ctx.enter_context(tc.tile_pool(name="lowT", bufs=2))
    smodpool = ctx.enter_context(tc.tile_pool(name="smod", bufs=4))
    opool = ctx.enter_context(tc.tile_pool(name="outsb", bufs=3))

    # HAM warm-up fodder: garbage bf16 matmuls while the first loads stream.
    wu_a = singles.tile([P, P], BF16)
    nc.vector.memset(wu_a[:], 1.0)
    wu_b = singles.tile([P, MS], BF16)
    nc.vector.memset(wu_b[:], 1.0)

    # ---- input DMA queue (sync ring), in first-use order ---------------
    # A single HWDGE ring sustains ~360 GB/s for 0.5-1 MiB transfers; the
    # order below is by first consumption: u/dataT0 pairs (rank stage 0),
    # w/ctx (s_mod), vT (output stage 0), then dataT1 (rank stage 1).
    # Output stores ride the gpsimd SWDGE ring so they never queue ahead
    # of loads.
    dataT_t = {0: []}
    dq = {}
    for q4 in range(4):
        dq[(0, q4)] = dTpool.tile(
            [P, 4, BT], F32R, tag=f"dataT0q{q4}", name=f"dataT0q{q4}"
        )
    dataT_t[0] = [dq[(0, q4)][:, j] for q4 in range(4) for j in range(4)]
    # batch-tile 1 is loaded as two 256-row halves so the final output
    # stage is gated by only half the remaining data.
    dqh = {}
    dataT_h = {}
    for h in range(2):
        for q4 in range(4):
            dqh[(h, q4)] = dTpool.tile(
                [P, 4, 256], F32R, tag=f"dataT1h{h}q{q4}", name=f"dataT1h{h}q{q4}"
            )
        dataT_h[h] = [dqh[(h, q4)][:, j] for q4 in range(4) for j in range(4)]
    u_t = []  # u_t[uq] = [P, 4, RANK] tile; chunk kc = u_t[kc//4][:, kc%4]
    for uq in range(4):
        ut = singles.tile([P, 4, RANK], F32R, name=f"uq{uq}")
        u_t.append(ut)

    nc.sync.dma_start(out=u_t[0][:], in_=ap_u[:, 0])
    nc.sync.dma_start(out=dq[(0, 0)][:], in_=ap_dataT[:, 0, :, 0:BT])
    w_sb = singles.tile([P, CC, RANK], F32R)
    nc.sync.dma_start(
        out=w_sb[:], in_=d_w.ap().rearrange("(cc p) r -> p cc r", p=P)
    )
    ctxT_t = {}
    ctxT_t[0] = cTpool.tile([P, CC, BT], F32R, tag="ctxT", name="ctxT0")
    nc.sync.dma_start(out=ctxT_t[0][:], in_=ap_ctxT[:, :, 0:BT])
    s_sb = singles.tile([P, RC], F32)
    nc.sync.dma_start(out=s_sb[:], in_=d_s.ap().rearrange("(rc p) -> p rc", p=P))
    bias2 = singles.tile([1, UNITS], F32R)
    nc.sync.dma_start(out=bias2[:], in_=d_bias.ap().rearrange("(a m) -> a m", a=1))
    for uq in (1, 2, 3):
        nc.sync.dma_start(out=u_t[uq][:], in_=ap_u[:, uq])
        nc.sync.dma_start(out=dq[(0, uq)][:], in_=ap_dataT[:, uq, :, 0:BT])
    vT_sb = singles.tile([P, RC, UNITS], F32R)
    nc.sync.dma_start(out=vT_sb[:, 0], in_=ap_vT[:, 0])
    nc.sync.dma_start(out=vT_sb[:, 1], in_=ap_vT[:, 1])
    ctxT_t[1] = cTpool.tile([P, CC, BT], F32R, tag="ctxT", name="ctxT1")
    nc.sync.dma_start(out=ctxT_t[1][:], in_=ap_ctxT[:, :, BT:])
    for h in range(2):
        for q4 in range(4):
            nc.sync.dma_start(
                out=dqh[(h, q4)][:],
                in_=ap_dataT[:, q4, :, BT + h * 256 : BT + (h + 1) * 256],
            )

    ones_f = singles.tile([1, P], F32)
    nc.vector.memset(ones_f[:], 2.0)
    ones = singles.tile([1, P], F32R)
    nc.vector.tensor_copy(out=ones[:], in_=ones_f[:])

    # ---- HAM warm-up ---------------------------------------------------
    wu_ps = o_psum.tile([P, MS], F32, tag="po", name="wu_ps")
    for _ in range(N_WARMUP_MM):
        nc.tensor.matmul(wu_ps[:], lhsT=wu_a[:], rhs=wu_b[:], start=True, stop=True)

    # ---- compute stages ------------------------------------------------
    lowT_t = {}
    pd_t = {}
    smod_t = {}

    def emit_warm_keepers(n):
        """No-dep bf16 matmuls that keep the HAM activity monitor above
        its throttle threshold while real matmuls are DMA-paced."""
        for _ in range(n):
            nc.tensor.matmul(
                wu_ps[:], lhsT=wu_a[:], rhs=wu_b[:], start=True, stop=True
            )

    def emit_rank_mms(key, chunks, width, kc_lo, kc_hi, keepers=False):
        """mm1T k-chunks [kc_lo, kc_hi) for both rank chunks."""
        if kc_lo == 0:
            pd_t[key] = [
                du_psum.tile([P, width], F32, tag="pd", name="pd")
                for _ in range(RC)
            ]
        for kc in range(kc_lo, kc_hi):
            for rc in range(RC):
                nc.tensor.matmul(
                    pd_t[key][rc][:],
                    lhsT=u_t[kc // 4][:, kc % 4, rc * P : (rc + 1) * P],
                    rhs=chunks[kc],
                    start=(kc == 0),
                    stop=(kc == KC - 1),
                )
            if keepers and kc % 2 == 1:
                emit_warm_keepers(2)

    def emit_smod(bt):
        """ctx @ w matmuls + s-add; independent of the data stream."""
        smod_t[bt] = []
        for rc in range(RC):
            ps = s_psum.tile([P, BT], F32, tag="ps", name="ps")
            for cc in range(CC):
                nc.tensor.matmul(
                    ps[:],
                    lhsT=w_sb[:, cc, rc * P : (rc + 1) * P],
                    rhs=ctxT_t[bt][:, cc, :],
                    start=(cc == 0),
                    stop=(cc == CC - 1),
                )
            smod = smodpool.tile([P, BT], F32, tag="smod", name="smod")
            nc.scalar.add(smod[:], ps[:], add=s_sb[:, rc : rc + 1])
            smod_t[bt].append(smod)

    def emit_mul(key, bt, width, off=0):
        """lowT = pd * smod on the vector engine."""
        lowT_t[key] = lowpool.tile(
            [P, RC, width], F32R, tag=f"lowT{width}", name="lowT"
        )
        for rc in range(RC):
            nc.vector.tensor_mul(
                out=lowT_t[key][:, rc, :],
                in0=pd_t[key][rc][:],
                in1=smod_t[bt][rc][:, off : off + width],
            )

    def emit_out_stage(key, row0, bc, fine_stores=False, store_engine=None, split_store=False):
        """out[b, :] = relu(low @ v.T + 2*bias) for one 128-row chunk.

        All four 512-wide PSUM groups stay open at once and the matmuls
        are ordered rc-major so consecutive matmuls reuse the same
        stationary operand; ReLU evacuation alternates between the
        scalar and vector engines.
        """
        lowT = lowT_t[key]
        osb = opool.tile([P, UNITS], F32, tag="osb", name="osb")
        pos = [o_psum.tile([P, MS], F32, tag="po", name="po") for _ in range(NMS)]
        for rc in range(RC):
            for ms in range(NMS):
                nc.tensor.matmul(
                    pos[ms][:],
                    lhsT=lowT[:, rc, bc * P : (bc + 1) * P],
                    rhs=vT_sb[:, rc, ms * MS : (ms + 1) * MS],
                    start=(rc == 0),
                    stop=False,
                )
        for ms in range(NMS):
            nc.tensor.matmul(
                pos[ms][:],
                lhsT=ones[:],
                rhs=bias2[:, ms * MS : (ms + 1) * MS],
                start=False,
                stop=True,
            )
        rows = slice(row0 + bc * P, row0 + (bc + 1) * P)
        eng = store_engine if store_engine is not None else nc.gpsimd
        for ms in range(NMS):
            sl = slice(ms * MS, (ms + 1) * MS)
            if ms % 2 == 0:
                nc.scalar.activation(
                    osb[:, sl], pos[ms][:], mybir.ActivationFunctionType.Relu
                )
            else:
                nc.vector.tensor_relu(out=osb[:, sl], in_=pos[ms][:])
            if fine_stores:
                seng = nc.sync if ms % 2 == 0 else nc.gpsimd
                seng.dma_start(out=d_out.ap()[rows, sl], in_=osb[:, sl])
        if not fine_stores:
            if split_store:
                h = UNITS // 2
                nc.gpsimd.dma_start(out=d_out.ap()[rows, :h], in_=osb[:, :h])
                nc.sync.dma_start(out=d_out.ap()[rows, h:], in_=osb[:, h:])
            else:
                eng.dma_start(out=d_out.ap()[rows, :], in_=osb[:])

    # Software pipeline, PE emission ordered to match DMA arrival order.
    emit_rank_mms(0, dataT_t[0], BT, 0, 4, keepers=True)
    emit_rank_mms(0, dataT_t[0], BT, 4, 8, keepers=True)
    emit_smod(0)
    emit_rank_mms(0, dataT_t[0], BT, 8, 12, keepers=True)
    emit_rank_mms(0, dataT_t[0], BT, 12, 16, keepers=True)
    emit_mul(0, 0, BT)
    emit_out_stage(0, 0, 0)
    emit_out_stage(0, 0, 1)
    emit_smod(1)
    emit_out_stage(0, 0, 2)
    emit_out_stage(0, 0, 3)
    emit_rank_mms("1a", dataT_h[0], 256, 0, 8)
    emit_rank_mms("1a", dataT_h[0], 256, 8, 16)
    emit_mul("1a", 1, 256, off=0)
    emit_out_stage("1a", BT, 0, split_store=True)
    emit_rank_mms("1b", dataT_h[1], 256, 0, 8)
    emit_out_stage("1a", BT, 1, split_store=True)
    emit_rank_mms("1b", dataT_h[1], 256, 8, 16)
    emit_mul("1b", 1, 256, off=256)
    emit_out_stage("1b", BT + 256, 0, split_store=True)
    emit_out_stage("1b", BT + 256, 1, fine_stores=True)


_CACHE = {}


def build():
    if "nc" in _CACHE:
        return _CACHE["nc"]
    nc = bacc.Bacc("TRN2", target_bir_lowering=False, debug=False)
    with tile.TileContext(nc) as tc, ExitStack() as ctx:
        _emit(nc, tc, ctx)
    nc.compile()
    _CACHE["nc"] = nc
    return nc


def make_in_maps(data, context, u, s, v, w, bias):
    u = np.ascontiguousarray(np.asarray(u, dtype=np.float32))
    s = np.ascontiguousarray(np.asarray(s, dtype=np.float32))
    vT = np.ascontiguousarray(np.asarray(v, dtype=np.float32).T)
    w = np.ascontiguousarray(np.asarray(w, dtype=np.float32))
    bias = np.ascontiguousarray(np.asarray(bias, dtype=np.float32))
    in_maps = []
    for c in range(NCORES):
        sl = slice(c * NB, (c + 1) * NB)
        in_maps.append(
            {
                "dataT": np.ascontiguousarray(np.asarray(data[sl], dtype=np.float32).T),
                "ctxT": np.ascontiguousarray(
                    np.asarray(context[sl], dtype=np.float32).T
                ),
                "u": u,
                "s": s,
                "vT": vT,
                "w": w,
                "bias": bias,
            }
        )
    return in_maps


def kernel(data, context, u, s, v, w, bias):
    nc = build()
    in_maps = make_in_maps(data, context, u, s, v, w, bias)
    res = run_bass_kernel_spmd(nc, in_maps, core_ids=list(range(NCORES)))
    return np.concatenate([r["out"] for r in res.results], axis=0)



# revision 4
# speedup vs baseline: 1.5492x; 1.5492x over previous
"""Trainium2 Bass kernel for nn_CADense (context-adaptive low-rank dense layer).

Computes, for the full batch:
    s_mod = s + context @ w          # [B, R]
    low   = (data @ u) * s_mod       # [B, R]
    out   = relu(low @ v.T + 2*bias) # [B, UNITS]

Sharding: data-parallel over batch across 8 NeuronCores; u/s/v/w/bias
replicated. Each core runs the same Bass program on its 1024-row shard.

All heavy streams are bf16: inputs are downcast host-side into pre-tiled,
fully-contiguous per-DMA slabs (partition-major, 4KB contiguous per
partition line), and the output is stored bf16 and upcast host-side.
This halves HBM traffic vs f32 (11.8 MB/core) and runs the PE at
1 cycle/row with hardware fast-weight-load, so DMA (~33 us) and PE
(~31 us) land together at the roofline ridge.

Compute per 512-row batch tile, in the transposed domain:
    pd[r, b]   = (u.T @ data.T)[r, b]          (16 k-chunk accumulation)
    smod[r, b] = s[r] + (w.T @ ctx.T)[r, b]    (4 c-chunk accumulation)
    lowT       = pd * smod                      (DVE, bf16 out)
    out[b, :]  = relu(lowT.T @ v.T)             (per 128-row chunk)
ReLU evacuation of the output PSUM alternates scalar/vector engines and
stores ride the gpsimd SWDGE ring; loads are split across the sync and
scalar HWDGE rings. bias is all-zero in this model configuration; a
separate program variant folds nonzero bias in as K=1 rank-1 matmuls.
"""

import os
import sys
from contextlib import ExitStack

import numpy as np

try:
    import ml_dtypes
except ImportError:  # pragma: no cover
    ml_dtypes = None


def _ensure_concourse():
    try:
        import concourse  # noqa: F401
    except ImportError:
        for p in ("/opt/trn_rl_repo", "/root/.axon_site/_ro/trn_rl_repo"):
            if os.path.isdir(p) and p not in sys.path:
                sys.path.insert(0, p)


_ensure_concourse()

import concourse.tile as tile  # noqa: E402
from concourse import bacc, mybir  # noqa: E402
from concourse.bass_utils import run_bass_kernel_spmd  # noqa: E402

if ml_dtypes is None:
    import ml_dtypes  # noqa: E402  (bundled with concourse deps)

NCORES = 8
B, N_IN, UNITS, RANK, CCTX = 8192, 2048, 2048, 256, 512
NB = B // NCORES  # batch rows per core
P = 128
BT = 512  # batch tile (free dim of T-domain matmuls)
NBT = NB // BT  # batch tiles per core
KC = N_IN // P  # 16 contraction chunks for data @ u
CC = CCTX // P  # 4 contraction chunks for context @ w
RC = RANK // P  # 2 rank chunks
MS = 512  # output units slice width
NMS = UNITS // MS  # 4 unit slices
NQ = KC // 4  # dataT slab count per batch tile (4 k-chunks each)
N_WARMUP_MM = 12

F32 = mybir.dt.float32
BF16 = mybir.dt.bfloat16
BF16_NP = ml_dtypes.bfloat16


def _emit(nc, tc, ctx, with_bias):
    # Host-pretiled bf16 slabs; every DMA source is fully contiguous.
    d_dataT = {
        (q, t): nc.dram_tensor(f"dataT{q}_{t}", [P, 4 * BT], BF16, kind="ExternalInput")
        for q in range(NQ)
        for t in range(NBT)
    }
    d_ctxT = {
        t: nc.dram_tensor(f"ctxT{t}", [P, CC * BT], BF16, kind="ExternalInput")
        for t in range(NBT)
    }
    d_u = {
        uq: nc.dram_tensor(f"u{uq}", [P, 4 * RANK], BF16, kind="ExternalInput")
        for uq in range(4)
    }
    d_s = nc.dram_tensor("s", [P, RC], F32, kind="ExternalInput")
    d_vT = nc.dram_tensor("vT", [P, RC * UNITS], BF16, kind="ExternalInput")
    d_w = nc.dram_tensor("w", [P, CC * RANK], BF16, kind="ExternalInput")
    d_out = nc.dram_tensor("out", [NB, UNITS], BF16, kind="ExternalOutput")
    if with_bias:
        d_bias = nc.dram_tensor("bias2", [1, UNITS], BF16, kind="ExternalInput")

    singles = ctx.enter_context(tc.tile_pool(name="singles", bufs=1))
    du_psum = ctx.enter_context(tc.tile_pool(name="du_psum", bufs=2, space="PSUM"))
    s_psum = ctx.enter_context(tc.tile_pool(name="s_psum", bufs=2, space="PSUM"))
    o_psum = ctx.enter_context(tc.tile_pool(name="o_psum", bufs=4, space="PSUM"))
    dTpool = ctx.enter_context(tc.tile_pool(name="dataT", bufs=1))
    cTpool = ctx.enter_context(tc.tile_pool(name="ctxT", bufs=2))
    lowpool = ctx.enter_context(tc.tile_pool(name="lowT", bufs=2))
    smodpool = ctx.enter_context(tc.tile_pool(name="smod", bufs=4))
    opool = ctx.enter_context(tc.tile_pool(name="outsb", bufs=3))

    # HAM warm-up fodder while the first loads stream.
    wu_a = singles.tile([P, P], BF16)
    nc.vector.memset(wu_a[:], 1.0)
    wu_b = singles.tile([P, 256], BF16)
    nc.vector.memset(wu_b[:], 1.0)

    # ---- input DMA queues, in first-use order --------------------------
    # sync ring: u / dataT / vT (heavy, compute-pacing);
    # scalar ring: w / ctxT / s / bias (light, needed early for smod).
    u_t = [singles.tile([P, 4 * RANK], BF16, name=f"uq{uq}") for uq in range(4)]
    dT_t = {
        (q, t): dTpool.tile([P, 4 * BT], BF16, tag=f"dT{q}_{t}", name=f"dT{q}_{t}")
        for q in range(NQ)
        for t in range(NBT)
    }
    w_sb = singles.tile([P, CC * RANK], BF16)
    ctxT_t = {t: cTpool.tile([P, CC * BT], BF16, tag="ctxT", name=f"ctxT{t}") for t in range(NBT)}
    s_sb = singles.tile([P, RC], F32)
    vT_sb = singles.tile([P, RC * UNITS], BF16)

    nc.scalar.dma_start(out=w_sb[:], in_=d_w.ap())
    nc.scalar.dma_start(out=ctxT_t[0][:], in_=d_ctxT[0].ap())
    nc.scalar.dma_start(out=s_sb[:], in_=d_s.ap())
    nc.scalar.dma_start(out=ctxT_t[1][:], in_=d_ctxT[1].ap())
    nc.scalar.dma_start(out=vT_sb[:], in_=d_vT.ap())
    if with_bias:
        bias2 = singles.tile([1, UNITS], BF16)
        nc.scalar.dma_start(out=bias2[:], in_=d_bias.ap())
        ones = singles.tile([1, P], BF16)
        nc.vector.memset(ones[:], 2.0)

    nc.sync.dma_start(out=u_t[0][:], in_=d_u[0].ap())
    nc.sync.dma_start(out=dT_t[(0, 0)][:], in_=d_dataT[(0, 0)].ap())
    nc.sync.dma_start(out=u_t[1][:], in_=d_u[1].ap())
    nc.sync.dma_start(out=dT_t[(1, 0)][:], in_=d_dataT[(1, 0)].ap())
    nc.sync.dma_start(out=u_t[2][:], in_=d_u[2].ap())
    nc.sync.dma_start(out=dT_t[(2, 0)][:], in_=d_dataT[(2, 0)].ap())
    nc.sync.dma_start(out=u_t[3][:], in_=d_u[3].ap())
    nc.sync.dma_start(out=dT_t[(3, 0)][:], in_=d_dataT[(3, 0)].ap())
    for q in range(NQ):
        nc.sync.dma_start(out=dT_t[(q, 1)][:], in_=d_dataT[(q, 1)].ap())

    # ---- HAM warm-up ---------------------------------------------------
    wu_ps = o_psum.tile([P, MS], F32, tag="po", name="wu_ps")
    for _ in range(N_WARMUP_MM):
        nc.tensor.matmul(wu_ps[:, 0:256], lhsT=wu_a[:], rhs=wu_b[:], start=True, stop=True)

    # ---- compute stages ------------------------------------------------
    pd_t = {}
    smod_t = {}
    lowT_t = {}

    def emit_rank_mms(t, q):
        """mm1: pd[rc] += u_chunk.T @ dataT_chunk for k-chunks of slab q."""
        if q == 0:
            pd_t[t] = [
                du_psum.tile([P, BT], F32, tag="pd", name=f"pd{t}_{rc}")
                for rc in range(RC)
            ]
        for j in range(4):
            kc = q * 4 + j
            for rc in range(RC):
                nc.tensor.matmul(
                    pd_t[t][rc][:],
                    lhsT=u_t[q][:, j * RANK + rc * P : j * RANK + (rc + 1) * P],
                    rhs=dT_t[(q, t)][:, j * BT : (j + 1) * BT],
                    start=(kc == 0),
                    stop=(kc == KC - 1),
                )

    def emit_smod(t):
        """smod[rc] = s + ctx @ w ; independent of the data stream."""
        smod_t[t] = []
        for rc in range(RC):
            ps = s_psum.tile([P, BT], F32, tag="ps", name=f"ps{t}_{rc}")
            for cc in range(CC):
                nc.tensor.matmul(
                    ps[:],
                    lhsT=w_sb[:, cc * RANK + rc * P : cc * RANK + (rc + 1) * P],
                    rhs=ctxT_t[t][:, cc * BT : (cc + 1) * BT],
                    start=(cc == 0),
                    stop=(cc == CC - 1),
                )
            smod = smodpool.tile([P, BT], F32, tag="smod", name=f"smod{t}_{rc}")
            nc.scalar.add(smod[:], ps[:], add=s_sb[:, rc : rc + 1])
            smod_t[t].append(smod)

    def emit_mul(t, bc):
        """lowT chunk bc = pd * smod on the vector engine (bf16 out)."""
        if bc == 0:
            lowT_t[t] = lowpool.tile([P, RC * BT], BF16, tag="lowT", name=f"lowT{t}")
        cols = slice(bc * P, (bc + 1) * P)
        for rc in range(RC):
            nc.vector.tensor_mul(
                out=lowT_t[t][:, rc * BT + bc * P : rc * BT + (bc + 1) * P],
                in0=pd_t[t][rc][:, cols],
                in1=smod_t[t][rc][:, cols],
            )

    def emit_out_stage(t, bc):
        """out rows = relu(low @ v.T [+ 2*bias]) for one 128-row chunk."""
        lowT = lowT_t[t]
        osb = opool.tile([P, UNITS], BF16, tag="osb", name=f"osb{t}_{bc}")
        for ms in range(NMS):
            po = o_psum.tile([P, MS], F32, tag="po", name=f"po{t}_{bc}_{ms}")
            for rc in range(RC):
                nc.tensor.matmul(
                    po[:],
                    lhsT=lowT[:, rc * BT + bc * P : rc * BT + (bc + 1) * P],
                    rhs=vT_sb[:, rc * UNITS + ms * MS : rc * UNITS + (ms + 1) * MS],
                    start=(rc == 0),
                    stop=(rc == RC - 1) and not with_bias,
                )
            if with_bias:
                nc.tensor.matmul(
                    po[:],
                    lhsT=ones[:],
                    rhs=bias2[:, ms * MS : (ms + 1) * MS],
                    start=False,
                    stop=True,
                )
            sl = slice(ms * MS, (ms + 1) * MS)
            if ms % 2 == 0:
                nc.scalar.activation(
                    osb[:, sl], po[:], mybir.ActivationFunctionType.Relu
                )
            else:
                nc.vector.tensor_relu(out=osb[:, sl], in_=po[:])
        rows = slice(t * BT + bc * P, t * BT + (bc + 1) * P)
        nc.gpsimd.dma_start(out=d_out.ap()[rows, :], in_=osb[:])

    # Software pipeline, PE emission ordered to match DMA arrival order.
    emit_rank_mms(0, 0)
    emit_smod(0)
    emit_rank_mms(0, 1)
    emit_rank_mms(0, 2)
    emit_rank_mms(0, 3)
    emit_smod(1)  # fills the PE while DVE runs mul(0, 0)
    emit_mul(0, 0)
    for bc in range(4):
        if bc < 3:
            emit_mul(0, bc + 1)
        emit_out_stage(0, bc)
    for q in range(NQ):
        emit_rank_mms(1, q)
    emit_mul(1, 0)
    for bc in range(4):
        if bc < 3:
            emit_mul(1, bc + 1)
        emit_out_stage(1, bc)


_CACHE = {}


def build(with_bias=False):
    key = ("nc", with_bias)
    if key in _CACHE:
        return _CACHE[key]
    nc = bacc.Bacc("TRN2", target_bir_lowering=False, debug=False)
    with tile.TileContext(nc) as tc, ExitStack() as ctx:
        _emit(nc, tc, ctx, with_bias)
    nc.compile()
    _CACHE[key] = nc
    return nc


def make_in_maps(data, context, u, s, v, w, bias, with_bias=False):
    bf = BF16_NP
    u = np.asarray(u, dtype=np.float32)
    s = np.asarray(s, dtype=np.float32)
    v = np.asarray(v, dtype=np.float32)
    w = np.asarray(w, dtype=np.float32)
    data = np.asarray(data, dtype=np.float32)
    context = np.asarray(context, dtype=np.float32)

    # u[(uq*4+j)*128+p, r] -> u_slab[uq][p, j*RANK+r]
    u_sl = u.reshape(4, 4, P, RANK).transpose(0, 2, 1, 3).reshape(4, P, 4 * RANK)
    u_sl = u_sl.astype(bf)
    # v[m, rc*128+p] -> vT_slab[p, rc*UNITS+m]
    vT_sl = v.reshape(UNITS, RC, P).transpose(2, 1, 0).reshape(P, RC * UNITS).astype(bf)
    # w[cc*128+p, r] -> w_slab[p, cc*RANK+r]
    w_sl = w.reshape(CC, P, RANK).transpose(1, 0, 2).reshape(P, CC * RANK).astype(bf)
    # s[rc*128+p] -> s_slab[p, rc]
    s_sl = np.ascontiguousarray(s.reshape(RC, P).T)

    shared = {"s": s_sl, "vT": vT_sl, "w": w_sl}
    for uq in range(4):
        shared[f"u{uq}"] = u_sl[uq]
    if with_bias:
        shared["bias2"] = (2.0 * bias.astype(np.float32)).reshape(1, UNITS).astype(bf)

    in_maps = []
    for c in range(NCORES):
        sl = slice(c * NB, (c + 1) * NB)
        ds = data[sl]
        cs = context[sl]
        # data[t*BT+b, (q*4+j)*128+p] -> dataT_slab[q][t][p, j*BT+b]
        dt_sl = (
            ds.reshape(NBT, BT, NQ, 4, P)
            .transpose(2, 0, 4, 3, 1)
            .reshape(NQ, NBT, P, 4 * BT)
            .astype(bf)
        )
        # context[t*BT+b, cc*128+p] -> ctxT_slab[t][p, cc*BT+b]
        ct_sl = (
            cs.reshape(NBT, BT, CC, P)
            .transpose(0, 3, 2, 1)
            .reshape(NBT, P, CC * BT)
            .astype(bf)
        )
        m = dict(shared)
        for q in range(NQ):
            for t in range(NBT):
                m[f"dataT{q}_{t}"] = dt_sl[q, t]
        for t in range(NBT):
            m[f"ctxT{t}"] = ct_sl[t]
        in_maps.append(m)
    return in_maps


def kernel(data, context, u, s, v, w, bias):
    with_bias = bool(np.any(np.asarray(bias)))
    nc = build(with_bias)
    in_maps = make_in_maps(data, context, u, s, v, w, bias, with_bias=with_bias)
    res = run_bass_kernel_spmd(nc, in_maps, core_ids=list(range(NCORES)))
    return np.concatenate(
        [np.asarray(r["out"]).astype(np.float32) for r in res.results], axis=0
    )


# revision 8
# speedup vs baseline: 1.5622x; 1.0084x over previous
"""Trainium2 Bass kernel for nn_CADense (context-adaptive low-rank dense layer).

Computes, for the full batch:
    s_mod = s + context @ w          # [B, R]
    low   = (data @ u) * s_mod       # [B, R]
    out   = relu(low @ v.T + 2*bias) # [B, UNITS]

Sharding: data-parallel over batch across 8 NeuronCores; u/s/v/w/bias
replicated. Each core runs the same Bass program on its 1024-row shard.

All heavy streams are bf16: inputs are downcast host-side into pre-tiled,
fully-contiguous per-DMA slabs (partition-major, 4KB contiguous per
partition line), and the output is stored bf16 and upcast host-side.
This halves HBM traffic vs f32 (11.8 MB/core) and runs the PE at
1 cycle/row with hardware fast-weight-load, so DMA (~33 us) and PE
(~31 us) land together at the roofline ridge.

Compute per 512-row batch tile, in the transposed domain:
    pd[r, b]   = (u.T @ data.T)[r, b]          (16 k-chunk accumulation)
    smod[r, b] = s[r] + (w.T @ ctx.T)[r, b]    (4 c-chunk accumulation)
    lowT       = pd * smod                      (DVE, bf16 out)
    out[b, :]  = relu(lowT.T @ v.T)             (per 128-row chunk)
ReLU evacuation of the output PSUM alternates scalar/vector engines and
stores ride the gpsimd SWDGE ring; loads are split across the sync and
scalar HWDGE rings. bias is all-zero in this model configuration; a
separate program variant folds nonzero bias in as K=1 rank-1 matmuls.
"""

import os
import sys
from contextlib import ExitStack

import numpy as np

try:
    import ml_dtypes
except ImportError:  # pragma: no cover
    ml_dtypes = None


def _ensure_concourse():
    try:
        import concourse  # noqa: F401
    except ImportError:
        for p in ("/opt/trn_rl_repo", "/root/.axon_site/_ro/trn_rl_repo"):
            if os.path.isdir(p) and p not in sys.path:
                sys.path.insert(0, p)


_ensure_concourse()

import concourse.tile as tile  # noqa: E402
from concourse import bacc, mybir  # noqa: E402
from concourse.bass_utils import run_bass_kernel_spmd  # noqa: E402

if ml_dtypes is None:
    import ml_dtypes  # noqa: E402  (bundled with concourse deps)

NCORES = 8
B, N_IN, UNITS, RANK, CCTX = 8192, 2048, 2048, 256, 512
NB = B // NCORES  # batch rows per core
P = 128
BT = 512  # batch tile (free dim of T-domain matmuls)
NBT = NB // BT  # batch tiles per core
KC = N_IN // P  # 16 contraction chunks for data @ u
CC = CCTX // P  # 4 contraction chunks for context @ w
RC = RANK // P  # 2 rank chunks
MS = 512  # output units slice width
NMS = UNITS // MS  # 4 unit slices
NQ = KC // 4  # dataT slab count per batch tile (4 k-chunks each)
N_WARMUP_MM = 12

F32 = mybir.dt.float32
BF16 = mybir.dt.bfloat16
BF16_NP = ml_dtypes.bfloat16


def _emit(nc, tc, ctx, with_bias):
    # Host-pretiled bf16 slabs; every DMA source is fully contiguous.
    # (0,0) is split into two half-slabs so the PE can start sooner.
    d_dataT = {
        (q, t): nc.dram_tensor(f"dataT{q}_{t}", [P, 4 * BT], BF16, kind="ExternalInput")
        for q in range(NQ)
        for t in range(NBT)
        if (q, t) != (0, 0)
    }
    d_dT00a = nc.dram_tensor("dataT0_0a", [P, 2 * BT], BF16, kind="ExternalInput")
    d_dT00b = nc.dram_tensor("dataT0_0b", [P, 2 * BT], BF16, kind="ExternalInput")
    d_ctxT = {
        t: nc.dram_tensor(f"ctxT{t}", [P, CC * BT], BF16, kind="ExternalInput")
        for t in range(NBT)
    }
    d_u = {
        uq: nc.dram_tensor(f"u{uq}", [P, 4 * RANK], BF16, kind="ExternalInput")
        for uq in range(4)
    }
    d_s = nc.dram_tensor("s", [P, RC], F32, kind="ExternalInput")
    d_vT = nc.dram_tensor("vT", [P, RC * UNITS], BF16, kind="ExternalInput")
    d_w = nc.dram_tensor("w", [P, CC * RANK], BF16, kind="ExternalInput")
    d_out = nc.dram_tensor("out", [NB, UNITS], BF16, kind="ExternalOutput")
    if with_bias:
        d_bias = nc.dram_tensor("bias2", [1, UNITS], BF16, kind="ExternalInput")

    singles = ctx.enter_context(tc.tile_pool(name="singles", bufs=1))
    du_psum = ctx.enter_context(tc.tile_pool(name="du_psum", bufs=2, space="PSUM"))
    s_psum = ctx.enter_context(tc.tile_pool(name="s_psum", bufs=2, space="PSUM"))
    o_psum = ctx.enter_context(tc.tile_pool(name="o_psum", bufs=4, space="PSUM"))
    dTpool = ctx.enter_context(tc.tile_pool(name="dataT", bufs=1))
    cTpool = ctx.enter_context(tc.tile_pool(name="ctxT", bufs=2))
    lowpool = ctx.enter_context(tc.tile_pool(name="lowT", bufs=2))
    smodpool = ctx.enter_context(tc.tile_pool(name="smod", bufs=4))
    opool = ctx.enter_context(tc.tile_pool(name="outsb", bufs=3))

    # HAM warm-up fodder while the first loads stream.
    wu_a = singles.tile([P, P], BF16)
    nc.vector.memset(wu_a[:], 1.0)
    wu_b = singles.tile([P, 256], BF16)
    nc.vector.memset(wu_b[:], 1.0)

    # ---- input DMA queues ----------------------------------------------
    # Both HWDGE rings front-load the bytes that gate the first mm2
    # (u, dataT bt0, w, ctx0); everything else queues behind them.
    u_t = [singles.tile([P, 4 * RANK], BF16, name=f"uq{uq}") for uq in range(4)]
    dT_t = {
        (q, t): dTpool.tile([P, 4 * BT], BF16, tag=f"dT{q}_{t}", name=f"dT{q}_{t}")
        for q in range(NQ)
        for t in range(NBT)
    }
    w_sb = singles.tile([P, CC * RANK], BF16)
    ctxT_t = {t: cTpool.tile([P, CC * BT], BF16, tag="ctxT", name=f"ctxT{t}") for t in range(NBT)}
    s_sb = singles.tile([P, RC], F32)
    vT_sb = singles.tile([P, RC * UNITS], BF16)

    # sync ring: q0/q1 of bt0, then ctx1 and all of bt1.
    nc.sync.dma_start(out=u_t[0][:], in_=d_u[0].ap())
    nc.sync.dma_start(out=dT_t[(0, 0)][:, 0 : 2 * BT], in_=d_dT00a.ap())
    nc.sync.dma_start(out=dT_t[(0, 0)][:, 2 * BT : 4 * BT], in_=d_dT00b.ap())
    nc.sync.dma_start(out=u_t[1][:], in_=d_u[1].ap())
    nc.sync.dma_start(out=dT_t[(1, 0)][:], in_=d_dataT[(1, 0)].ap())
    nc.sync.dma_start(out=ctxT_t[1][:], in_=d_ctxT[1].ap())
    for q in range(NQ):
        nc.sync.dma_start(out=dT_t[(q, 1)][:], in_=d_dataT[(q, 1)].ap())

    # scalar ring: smod-0 inputs, then q2/q3 of bt0, then s and vT.
    nc.scalar.dma_start(out=w_sb[:], in_=d_w.ap())
    nc.scalar.dma_start(out=ctxT_t[0][:], in_=d_ctxT[0].ap())
    nc.scalar.dma_start(out=u_t[2][:], in_=d_u[2].ap())
    nc.scalar.dma_start(out=dT_t[(2, 0)][:], in_=d_dataT[(2, 0)].ap())
    nc.scalar.dma_start(out=u_t[3][:], in_=d_u[3].ap())
    nc.scalar.dma_start(out=dT_t[(3, 0)][:], in_=d_dataT[(3, 0)].ap())
    nc.scalar.dma_start(out=s_sb[:], in_=d_s.ap())
    nc.scalar.dma_start(out=vT_sb[:], in_=d_vT.ap())
    if with_bias:
        bias2 = singles.tile([1, UNITS], BF16)
        nc.scalar.dma_start(out=bias2[:], in_=d_bias.ap())
        ones = singles.tile([1, P], BF16)
        nc.vector.memset(ones[:], 2.0)

    # ---- HAM warm-up ---------------------------------------------------
    wu_ps = o_psum.tile([P, MS], F32, tag="po", name="wu_ps")
    for _ in range(N_WARMUP_MM):
        nc.tensor.matmul(wu_ps[:, 0:256], lhsT=wu_a[:], rhs=wu_b[:], start=True, stop=True)

    # ---- compute stages ------------------------------------------------
    pd_t = {}
    smod_t = {}
    lowT_t = {}

    def emit_rank_mms(t, q):
        """mm1: pd[rc] += u_chunk.T @ dataT_chunk for k-chunks of slab q."""
        if q == 0:
            pd_t[t] = [
                du_psum.tile([P, BT], F32, tag="pd", name=f"pd{t}_{rc}")
                for rc in range(RC)
            ]
        for j in range(4):
            kc = q * 4 + j
            for rc in range(RC):
                nc.tensor.matmul(
                    pd_t[t][rc][:],
                    lhsT=u_t[q][:, j * RANK + rc * P : j * RANK + (rc + 1) * P],
                    rhs=dT_t[(q, t)][:, j * BT : (j + 1) * BT],
                    start=(kc == 0),
                    stop=(kc == KC - 1),
                )

    def emit_smod(t):
        """smod[rc] = s + ctx @ w ; independent of the data stream."""
        smod_t[t] = []
        for rc in range(RC):
            ps = s_psum.tile([P, BT], F32, tag="ps", name=f"ps{t}_{rc}")
            for cc in range(CC):
                nc.tensor.matmul(
                    ps[:],
                    lhsT=w_sb[:, cc * RANK + rc * P : cc * RANK + (rc + 1) * P],
                    rhs=ctxT_t[t][:, cc * BT : (cc + 1) * BT],
                    start=(cc == 0),
                    stop=(cc == CC - 1),
                )
            smod = smodpool.tile([P, BT], F32, tag="smod", name=f"smod{t}_{rc}")
            nc.scalar.add(smod[:], ps[:], add=s_sb[:, rc : rc + 1])
            smod_t[t].append(smod)

    def emit_mul(t, bc):
        """lowT chunk bc = pd * smod on the vector engine (bf16 out)."""
        if bc == 0:
            lowT_t[t] = lowpool.tile([P, RC * BT], BF16, tag="lowT", name=f"lowT{t}")
        cols = slice(bc * P, (bc + 1) * P)
        for rc in range(RC):
            nc.vector.tensor_mul(
                out=lowT_t[t][:, rc * BT + bc * P : rc * BT + (bc + 1) * P],
                in0=pd_t[t][rc][:, cols],
                in1=smod_t[t][rc][:, cols],
            )

    def emit_out_stage(t, bc, store):
        """out rows = relu(low @ v.T [+ 2*bias]) for one 128-row chunk.

        store: engine for a whole-tile store, or a list of 4 engines for
        per-ms fine stores (used at the tail so the last store is small).
        """
        lowT = lowT_t[t]
        osb = opool.tile([P, UNITS], BF16, tag="osb", name=f"osb{t}_{bc}")
        rows = slice(t * BT + bc * P, t * BT + (bc + 1) * P)
        for ms in range(NMS):
            po = o_psum.tile([P, MS], F32, tag="po", name=f"po{t}_{bc}_{ms}")
            for rc in range(RC):
                nc.tensor.matmul(
                    po[:],
                    lhsT=lowT[:, rc * BT + bc * P : rc * BT + (bc + 1) * P],
                    rhs=vT_sb[:, rc * UNITS + ms * MS : rc * UNITS + (ms + 1) * MS],
                    start=(rc == 0),
                    stop=(rc == RC - 1) and not with_bias,
                )
            if with_bias:
                nc.tensor.matmul(
                    po[:],
                    lhsT=ones[:],
                    rhs=bias2[:, ms * MS : (ms + 1) * MS],
                    start=False,
                    stop=True,
                )
            sl = slice(ms * MS, (ms + 1) * MS)
            if ms % 2 == 0:
                nc.scalar.activation(
                    osb[:, sl], po[:], mybir.ActivationFunctionType.Relu
                )
            else:
                nc.vector.tensor_relu(out=osb[:, sl], in_=po[:])
            if isinstance(store, list):
                store[ms].dma_start(out=d_out.ap()[rows, sl], in_=osb[:, sl])
        if not isinstance(store, list):
            store.dma_start(out=d_out.ap()[rows, :], in_=osb[:])

    # Software pipeline: PE emission ordered by DMA arrival; bt1's rank
    # stage interleaves with bt0's output stage so the PE never waits on
    # the mul handoff, and stores alternate gpsimd/scalar rings (the
    # last two chunks fine-store over the by-then-idle sync ring).
    emit_rank_mms(0, 0)
    emit_rank_mms(0, 1)
    emit_smod(0)
    emit_rank_mms(0, 2)
    emit_rank_mms(0, 3)
    for bc in range(4):
        emit_mul(0, bc)
    emit_out_stage(0, 0, nc.gpsimd)
    emit_smod(1)
    emit_out_stage(0, 1, nc.scalar)
    emit_rank_mms(1, 0)
    emit_out_stage(0, 2, nc.gpsimd)
    emit_rank_mms(1, 1)
    emit_out_stage(0, 3, nc.scalar)
    emit_rank_mms(1, 2)
    emit_rank_mms(1, 3)
    for bc in range(4):
        emit_mul(1, bc)
    emit_out_stage(1, 0, nc.gpsimd)
    emit_out_stage(1, 1, nc.scalar)
    emit_out_stage(1, 2, [nc.scalar, nc.sync, nc.scalar, nc.sync])
    emit_out_stage(1, 3, [nc.sync, nc.scalar, nc.sync, nc.scalar])


_CACHE = {}


def build(with_bias=False):
    key = ("nc", with_bias)
    if key in _CACHE:
        return _CACHE[key]
    nc = bacc.Bacc("TRN2", target_bir_lowering=False, debug=False)
    with tile.TileContext(nc) as tc, ExitStack() as ctx:
        _emit(nc, tc, ctx, with_bias)
    nc.compile()
    _CACHE[key] = nc
    return nc


def make_in_maps(data, context, u, s, v, w, bias, with_bias=False):
    bf = BF16_NP
    u = np.asarray(u, dtype=np.float32)
    s = np.asarray(s, dtype=np.float32)
    v = np.asarray(v, dtype=np.float32)
    w = np.asarray(w, dtype=np.float32)
    data = np.asarray(data, dtype=np.float32)
    context = np.asarray(context, dtype=np.float32)

    # u[(uq*4+j)*128+p, r] -> u_slab[uq][p, j*RANK+r]
    u_sl = u.reshape(4, 4, P, RANK).transpose(0, 2, 1, 3).reshape(4, P, 4 * RANK)
    u_sl = u_sl.astype(bf)
    # v[m, rc*128+p] -> vT_slab[p, rc*UNITS+m]
    vT_sl = v.reshape(UNITS, RC, P).transpose(2, 1, 0).reshape(P, RC * UNITS).astype(bf)
    # w[cc*128+p, r] -> w_slab[p, cc*RANK+r]
    w_sl = w.reshape(CC, P, RANK).transpose(1, 0, 2).reshape(P, CC * RANK).astype(bf)
    # s[rc*128+p] -> s_slab[p, rc]
    s_sl = np.ascontiguousarray(s.reshape(RC, P).T)

    shared = {"s": s_sl, "vT": vT_sl, "w": w_sl}
    for uq in range(4):
        shared[f"u{uq}"] = u_sl[uq]
    if with_bias:
        shared["bias2"] = (2.0 * bias.astype(np.float32)).reshape(1, UNITS).astype(bf)

    in_maps = []
    for c in range(NCORES):
        sl = slice(c * NB, (c + 1) * NB)
        ds = data[sl]
        cs = context[sl]
        # data[t*BT+b, (q*4+j)*128+p] -> dataT_slab[q][t][p, j*BT+b]
        dt_sl = (
            ds.reshape(NBT, BT, NQ, 4, P)
            .transpose(2, 0, 4, 3, 1)
            .reshape(NQ, NBT, P, 4 * BT)
            .astype(bf)
        )
        # context[t*BT+b, cc*128+p] -> ctxT_slab[t][p, cc*BT+b]
        ct_sl = (
            cs.reshape(NBT, BT, CC, P)
            .transpose(0, 3, 2, 1)
            .reshape(NBT, P, CC * BT)
            .astype(bf)
        )
        m = dict(shared)
        for q in range(NQ):
            for t in range(NBT):
                if (q, t) == (0, 0):
                    m["dataT0_0a"] = np.ascontiguousarray(dt_sl[0, 0][:, : 2 * BT])
                    m["dataT0_0b"] = np.ascontiguousarray(dt_sl[0, 0][:, 2 * BT :])
                else:
                    m[f"dataT{q}_{t}"] = dt_sl[q, t]
        for t in range(NBT):
            m[f"ctxT{t}"] = ct_sl[t]
        in_maps.append(m)
    return in_maps


def kernel(data, context, u, s, v, w, bias):
    with_bias = bool(np.any(np.asarray(bias)))
    nc = build(with_bias)
    in_maps = make_in_maps(data, context, u, s, v, w, bias, with_bias=with_bias)
    res = run_bass_kernel_spmd(nc, in_maps, core_ids=list(range(NCORES)))
    return np.concatenate(
        [np.asarray(r["out"]).astype(np.float32) for r in res.results], axis=0
    )
